# revision 45
# baseline (speedup 1.0000x reference)
# Trainium2 Bass kernel for nn_CvtLstm: ConvLSTM cell with 4-branch,
# 4-head spatial attention. Data-parallel over batch N=32 across 8
# NeuronCores (4 samples per core); weights replicated to every core.
#
# Per-core layout: channels on partitions, flattened 16x16 spatial (256)
# on the free dim. conv3x3 = 9 shifted matmuls reading a zero-padded
# [128, 2, 18, 18] tile (borders zeroed by Pool-engine memset, not DMA).
# Attention scores are computed directly in the transposed [d, q] layout
# (lhsT = per-head k rows, K=32 row-partial matmuls at per-head PE row
# offsets); exp on the ACT engine with no max subtraction (scores lie in
# [-9, 8]), output in bf16. The PV product runs transposed: per (head,
# query-chunk) a K=128 (keys) x N=33 bf16 matmul whose rhs columns are
# [v_head | ones], yielding aT[q, ch] plus the softmax denominator Z as
# column 32; normalization is one strided reciprocal + one broadcast
# multiply on DVE, and two 128x128 bf16 PE transposes restore the
# channel-major layout consumed by the gate matmuls. This removes all
# SBUF-to-SBUF restack DMAs of the previous design (HWDGE descriptor
# processing is a serialized ~630ns/DMA device).
#
# Gates use tanh only (sigmoid(x) = 0.5 + 0.5*tanh(x/2), realized with
# the ACT scale input and algebraic folding into the cell update and
# 0.5*W_out), so the ACT engine never switches activation-function
# tables (exp/tanh/identity share the exp_and_others set).
#
# Hardware constraints honored: two row-partial matmuls at different PE
# row groups back-to-back fault the device; every score row-group
# transition is separated by full-row (K=128) matmuls -- the pipelined
# PV/transpose matmuls of earlier iterations or conv/gate filler
# matmuls. The neuron compiler rejects mixed 32/16-bit matmul inputs,
# f32r or strided transposes; transposes here are contiguous bf16.

import numpy as np

N, I, H, W = 32, 64, 16, 16
R, CM, A, HEADS, HC = 128, 128, 128, 4, 32
HW = H * W           # 256
S = 4                # samples per core
NCORES = 8

_CACHE = {}

# Matmuls with 16-bit operands each carry a standalone LDWEIGHTS
# instruction (~70ns of PE sequencer issue time); fp32r matmuls with
# moving size >= 256 run at the same 1 cycle/row and self-load their
# weights. So every N>=256 matmul uses f32r; only the tiny N=33 PV
# matmuls and the transposes (where f32r is 4x slower / rejected)
# stay bf16 unless PV_F32R below flips them.
PV_F32R = False


def _build_program():
    import contextlib
    import concourse.bacc as bacc
    import concourse.mybir as mybir
    import concourse.tile as tile

    F32 = mybir.dt.float32
    F32R = mybir.dt.float32r
    BF16 = mybir.dt.bfloat16
    AF = mybir.ActivationFunctionType
    PVDT = F32R if PV_F32R else BF16
    PVDT_P = F32 if PV_F32R else BF16

    nc = bacc.Bacc("TRN2", target_bir_lowering=False, debug=False)

    def dram(name, shape, dt=F32, kind="ExternalInput"):
        return nc.dram_tensor(name, list(shape), dt, kind=kind).ap()

    xin = dram("xin", [S, I, HW])
    hin = dram("hin", [S, R, HW])
    cin = dram("cin", [S, R, HW])
    winTd = dram("winT", [I, R])
    biasd = dram("biases", [128, 6])          # b_in | btok_eff(4) | bout
    wconvTd = dram("wconvT", [128, 2, 9, 128])
    wqkTd = dram("wqkT", [128, 2, 4, 128])
    wvTd = dram("wvT", [128, 2, 256])
    identd = dram("ident", [128, 128], BF16)
    wtokTd = dram("wtokT", [128, 4, 4, 128])
    wskipTd = dram("wskipT", [128, 4, 2, 128])
    woutTd = dram("woutT", [128, 128], BF16)  # pre-scaled by 0.5 (o-gate fold)
    yout = dram("yout", [S, R, HW], kind="ExternalOutput")

    QSRC = [0, 0, 1, 1]   # q source per branch: 0=xc, 1=hc
    KSRC = [0, 1, 0, 1]   # k/v source per branch
    BORDER = [3, 1, 2, 0]  # branch order (b3 = pure hc, earliest)

    with tile.TileContext(nc) as tc:
        with contextlib.ExitStack() as ctx:
            wpool = ctx.enter_context(tc.tile_pool(name="wts", bufs=1))
            sbA = ctx.enter_context(tc.tile_pool(name="sbA", bufs=2))
            sbB = ctx.enter_context(tc.tile_pool(name="sbB", bufs=2))
            stp = ctx.enter_context(tc.tile_pool(name="st", bufs=2, space="PSUM"))
            azp = ctx.enter_context(tc.tile_pool(name="az", bufs=1, space="PSUM"))
            pwp = ctx.enter_context(tc.tile_pool(name="pw", bufs=2, space="PSUM"))

            # ------------- input DMAs for pass 0 come first -------------
            # (zero borders via Pool memset; interiors by DMA; weights after)
            xt_pads = [None, None]
            h_pads = [None, None]
            x2s = [None, None]

            def emit_pads_dma(p):
                xt_pad = sbA.tile([128, 648], F32R, tag="xtpad", name="xtpad")
                h_pad = sbA.tile([128, 648], F32R, tag="hpad", name="hpad")
                x2 = sbA.tile([64, 2, 256], F32R, tag="x2", name="x2")
                nc.gpsimd.memset(h_pad.bitcast(F32), 0.0)
                nc.gpsimd.memset(xt_pad.bitcast(F32), 0.0)
                nc.sync.dma_start(
                    out=x2,
                    in_=xin[2 * p:2 * p + 2].rearrange("s c q -> c s q").bitcast(F32R))
                hv = h_pad.rearrange("p (s y x) -> p s y x", s=2, y=18, x=18)
                for s in range(2):
                    nc.sync.dma_start(
                        out=hv[:, s, 1:17, 1:17],
                        in_=hin[2 * p + s].rearrange(
                            "c (h w) -> c h w", h=16).bitcast(F32R))
                xt_pads[p], h_pads[p], x2s[p] = xt_pad, h_pad, x2

            # warm the exp_and_others ACT table immediately so the
            # 1.28us table load overlaps the input DMAs instead of the
            # first tanh on the prologue critical path
            warm = wpool.tile([1, 2], F32, tag="warm", name="warm")
            nc.vector.memset(warm, 0.0)
            nc.scalar.activation(out=warm, in_=warm, func=AF.Tanh)

            # weights; DMA issue order is the priority order: the hc-conv
            # weights and h interiors gate the longest prologue chain.
            def wload(name, src, shape, dt=F32R, defer=False, eng=None):
                t = wpool.tile(shape, dt, tag=name, name=name)
                if not defer:
                    (eng or nc.sync).dma_start(
                        out=t, in_=src.bitcast(dt) if dt == F32R else src)
                return t

            # bulk weights go through the Pool engine's SWDGE path: the
            # descriptor generation runs on the (idle) Pool engine instead
            # of the serialized ~625ns/DMA shared HWDGE, which stays free
            # for the latency-critical x/h/c input loads.
            wconvT_s = wload("wconvT", wconvTd, [128, 2, 9, 128], defer=True)
            emit_pads_dma(0)
            winT_s = wload("winT", winTd, [I, R])
            bias_s = wload("bias", biasd, [128, 6], F32)
            for t0 in (0, 3, 6):
                nc.gpsimd.dma_start(out=wconvT_s[:, 1, t0:t0 + 3, :],
                                    in_=wconvTd[:, 1, t0:t0 + 3, :].bitcast(F32R))
            wqkT_s = wload("wqkT", wqkTd, [128, 2, 4, 128], eng=nc.gpsimd)
            wvT_s = wload("wvT", wvTd, [128, 2, 256], eng=nc.gpsimd)
            ident_s = wload("ident", identd, [128, 128], BF16, eng=nc.gpsimd)
            nc.gpsimd.dma_start(out=wconvT_s[:, 0, :, :],
                                in_=wconvTd[:, 0, :, :].bitcast(F32R))
            wtokT_s = wload("wtokT", wtokTd, [128, 4, 4, 128], eng=nc.gpsimd)
            wskipT_s = wload("wskipT", wskipTd, [128, 4, 2, 128],
                             eng=nc.gpsimd)
            woutT_s = wload("woutT", woutTd, [128, 128], BF16, eng=nc.gpsimd)

            # ---------------- per-pass state ----------------
            xc_sb = [None, None]
            hc_sb = [None, None]
            q_sb = [[None] * 4, [None] * 4]
            k_sb = [[None] * 4, [None] * 4]
            vt_sb = [[[None, None], [None, None]],
                     [[None, None], [None, None]]]   # [p][s][c]
            a_all = [None, None]
            cprev_sb = [None, None]
            gate_sb = [[None] * 4, [None] * 4]       # p0 full-width
            gate_sbh = [[[None] * 2 for _ in range(4)] for _ in range(2)]

            def emit_xt(p):
                XT = pwp.tile([128, 512], F32, tag="pw", name="XT")
                nc.tensor.matmul(out=XT, lhsT=winT_s,
                                 rhs=x2s[p].rearrange("p s q -> p (s q)"),
                                 start=True, stop=True)
                xv = xt_pads[p].rearrange("p (s y x) -> p s y x", s=2, y=18, x=18)
                nc.scalar.activation(
                    out=xv[:, :, 1:17, 1:17],
                    in_=XT.rearrange("p (s h w) -> p s h w", s=2, h=16, w=16),
                    func=AF.Tanh, bias=bias_s[:, 0:1])

            def conv_chunks(p, src):
                """3x3 SAME conv via 9 shifted matmuls; returns 4 closures."""
                pad = xt_pads[p] if src == 0 else h_pads[p]
                pv = pad.rearrange("p (s y x) -> p s y x", s=2, y=18, x=18)
                state = {}

                def taps(t0, t1):
                    def go():
                        if t0 == 0:
                            state["CP"] = pwp.tile([128, 512], F32, tag="pw",
                                                   name="CP")
                        for t in range(t0, t1):
                            ky, kx = divmod(t, 3)
                            nc.tensor.matmul(out=state["CP"],
                                             lhsT=wconvT_s[:, src, t, :],
                                             rhs=pv[:, :, ky:ky + 16, kx:kx + 16],
                                             start=(t == 0), stop=(t == 8))
                    return go

                def fin():
                    dst = sbA.tile([128, 512], F32R,
                                   tag=("xc" if src == 0 else "hc"),
                                   name=("xc" if src == 0 else "hc"))
                    nc.vector.tensor_copy(dst, state["CP"])
                    if src == 0:
                        xc_sb[p] = dst
                    else:
                        hc_sb[p] = dst
                return [taps(0, 3), taps(3, 6), taps(6, 9), fin]

            def emit_conv(p, src):
                for f in conv_chunks(p, src):
                    f()

            def emit_q(p, b):
                srcq = xc_sb[p] if QSRC[b] == 0 else hc_sb[p]
                QB = pwp.tile([128, 512], F32, tag="pw", name="QB")
                nc.tensor.matmul(out=QB, lhsT=wqkT_s[:, 0, b, :], rhs=srcq,
                                 start=True, stop=True)
                q_sb[p][b] = sbB.tile([128, 512], F32R, tag=f"q{b}", name=f"q{b}")
                nc.vector.tensor_copy(q_sb[p][b], QB)

            def emit_k(p, b):
                srck = xc_sb[p] if KSRC[b] == 0 else hc_sb[p]
                KB = pwp.tile([128, 512], F32, tag="pw", name="KB")
                nc.tensor.matmul(out=KB, lhsT=wqkT_s[:, 1, b, :], rhs=srck,
                                 start=True, stop=True)
                k_sb[p][b] = sbB.tile([128, 512], F32R, tag=f"k{b}", name=f"k{b}")
                nc.vector.tensor_copy(k_sb[p][b], KB)

            def emit_qk(p, b):
                emit_q(p, b)
                emit_k(p, b)

            def emit_vt1(p, src, s, c):
                """vT tile piece [keys, b*132 + g*33 + (32 v | 1 one)] for the
                two branches fed by `src`, one (s, c) chunk (one pw alloc)."""
                src_sb = xc_sb[p] if src == 0 else hc_sb[p]
                sv = src_sb.rearrange("p (s c d) -> p s c d", s=2, c=2)
                if True:
                    vt = vt_sb[p][s][c]
                    if vt is None:
                        vt = sbB.tile([128, 528], PVDT, tag=f"vt{s}{c}",
                                      name=f"vt{s}{c}")
                        vt_sb[p][s][c] = vt
                        vt_ones = vt.rearrange(
                            "p (x u) -> p x u", u=33)[:, :, 32:33]
                        if PV_F32R:
                            vt_ones = vt_ones.bitcast(F32)
                        nc.gpsimd.memset(vt_ones, 1.0)
                    VT = pwp.tile([128, 512], F32, tag="pw", name="VT")
                    nc.tensor.matmul(out=VT[:, 0:256], lhsT=sv[:, s, c, :],
                                     rhs=wvT_s[:, src, :], start=True, stop=True)
                    for jb, b in enumerate((src, src + 2)):
                        dst = vt[:, b * 132:b * 132 + 132].rearrange(
                            "p (g u) -> p g u", u=33)[:, :, 0:32]
                        nc.vector.tensor_copy(
                            dst, VT[:, jb * 128:jb * 128 + 128].rearrange(
                                "p (g u) -> p g u", u=32))

            def emit_vt(p, src, s):
                for c in range(2):
                    emit_vt1(p, src, s, c)

            def emit_cprev(p):
                cprev_sb[p] = sbA.tile([128, 512], F32, tag="cprev", name="cprev")
                nc.sync.dma_start(
                    out=cprev_sb[p],
                    in_=cin[2 * p:2 * p + 2].rearrange("s c q -> c s q"))

            # ---------------- attention pipeline ----------------
            iters = ([(0, b, s) for b in BORDER for s in (0, 1)] +
                     [(1, b, s) for s in (0, 1) for b in BORDER])
            NIT = len(iters)
            az_t = [azp.tile([128, 512], F32, tag="az0", name="az0"),
                    azp.tile([128, 512], F32, tag="az1", name="az1")]
            pTs = [None] * NIT
            aTns = [None] * NIT
            pending_act = []   # deferred ACT emissions (gate tanh)

            def pv_lhs(pT, o):
                sl = pT[:, o:o + 128]
                return sl.bitcast(F32R) if PV_F32R else sl

            def pv_chunk(j, g):
                p, b, s = iters[j]
                az = az_t[j % 2]
                for qc in range(2):
                    for c in range(2):
                        nc.tensor.matmul(
                            out=az[:, (qc * 4 + g) * 33:(qc * 4 + g) * 33 + 33],
                            lhsT=pv_lhs(pTs[j], g * 512 + c * 256 +
                                        qc * 128),
                            rhs=vt_sb[p][s][c][:, b * 132 + g * 33:
                                               b * 132 + g * 33 + 33],
                            start=(c == 0), stop=(c == 1), skip_group_check=True)

            def norm(j):
                az = az_t[j % 2]
                rz = sbB.tile([128, 8], F32, tag="rz", name="rz")
                azv = az[:, 0:264].rearrange("p (x u) -> p x u", u=33)
                nc.vector.reciprocal_approx_fast(
                    out=rz.rearrange("p (x u) -> p x u", u=1),
                    in_=azv[:, :, 32:33])
                aTn = sbB.tile([128, 256], BF16, tag="atn", name="aTn")
                nc.vector.tensor_mul(
                    aTn.rearrange("p (x u) -> p x u", u=32),
                    azv[:, :, 0:32],
                    rz.rearrange("p (x u) -> p x u", u=1).to_broadcast(
                        (128, 8, 32)))
                aTns[j] = aTn

            def trs_copies(j):
                p, b, s = iters[j]
                spare = az_t[j % 2][:, 264:392].bitcast(BF16)
                if a_all[p] is None:
                    a_all[p] = sbA.tile([128, 2048], F32R, tag="aall",
                                        name="a_all")
                for qc in range(2):
                    nc.tensor.matmul(out=spare[:, qc * 128:qc * 128 + 128],
                                     lhsT=aTns[j][:, qc * 128:qc * 128 + 128],
                                     rhs=ident_s, is_transpose=True,
                                     skip_group_check=True)
                    slot = (b * 2 + s) * 256 + qc * 128
                    nc.vector.tensor_copy(a_all[p][:, slot:slot + 128],
                                          spare[:, qc * 128:qc * 128 + 128])

            def emit_iter(j, sep=None, fa=(), fb=()):
                """scores + exp for iteration j; separator slots carry the
                pipelined pv/transpose work of earlier iterations (all
                full-row, satisfying the row-group-transition rule).
                fa/fb: filler chunks emitted after exp(h0)/exp(h1) so a
                blocked filler never delays this iteration's scores."""
                p, b, s = iters[j]
                kv = k_sb[p][b].rearrange("p (s c d) -> p s c d", s=2, c=2)
                qv = q_sb[p][b].rearrange("p (s q) -> p s q", s=2)
                pT = sbB.tile([128, 2048], PVDT_P, tag="pt", name="pT")
                pTs[j] = pT

                def slot(si):
                    if sep is not None:
                        sep[si]()
                        return
                    if si == 0 and j >= 2:
                        trs_copies(j - 2)
                    if j >= 1:
                        pv_chunk(j - 1, si)
                        if si == 3:
                            norm(j - 1)

                for h in range(2):
                    ST = stp.tile([128, 1024], F32, tag="st", name="ST")
                    for gg in range(2):
                        g = 2 * h + gg
                        for c in range(2):
                            nc.tensor.matmul(
                                out=ST[:, gg * 512 + c * 256:
                                       gg * 512 + c * 256 + 256],
                                lhsT=kv[32 * g:32 * g + 32, s, c, :],
                                rhs=qv[32 * g:32 * g + 32, s, :],
                                start=True, stop=True, skip_group_check=True,
                                tile_position=(32 * g, 0))
                        slot(2 * h + gg)
                    nc.scalar.activation(out=pT[:, h * 1024:(h + 1) * 1024],
                                         in_=ST, func=AF.Exp)
                    if h == 0:
                        for f in pending_act:
                            f()
                        pending_act.clear()
                        for f in fa:
                            f()
                    else:
                        for f in fb:
                            f()

            # ---------------- gates / state / output ----------------
            gpack = {}

            gate_phase = {}

            def gate_mm(p, gi, s=None, pack=None, phases=None):
                """gate matmuls into PSUM; ACT tanh deferred via pending_act.
                s=None: full width (both samples); else 256-wide half. pack:
                (key, half) to place two 256-wide gates in one pw tile so the
                pool rotation is not serialized by the tanh reads."""
                if pack is None:
                    G = pwp.tile([128, 512], F32, tag="pw", name="G")
                    ofs = 0
                else:
                    key, half = pack
                    if key not in gpack:
                        gpack[key] = pwp.tile([128, 512], F32, tag="pw",
                                              name="G")
                    G = gpack[key]
                    ofs = half * 256
                w = 512 if s is None else 256
                Gs = G[:, ofs:ofs + w]
                av = a_all[p].rearrange("p (b s q) -> p b s q", b=4, s=2)

                def acc(phase):
                    # phase 0: skips + first branches (emittable before the
                    # last branches' attention finishes); 1,2: stragglers
                    first = phase != 0 or False
                    if phase == 0:
                        for si_, src in enumerate((xc_sb[p], hc_sb[p])):
                            rhs = (src if s is None
                                   else src[:, s * 256:s * 256 + 256])
                            nc.tensor.matmul(out=Gs,
                                             lhsT=wskipT_s[:, gi, si_, :],
                                             rhs=rhs, start=(si_ == 0),
                                             stop=False,
                                             skip_group_check=True)
                    bis = {0: (3, 1), 1: (2,), 2: (0,)}[phase]
                    for bi in bis:
                        rhs = (av[:, bi].rearrange("p s q -> p (s q)")
                               if s is None else av[:, bi, s, :])
                        nc.tensor.matmul(out=Gs, lhsT=wtokT_s[:, gi, bi, :],
                                         rhs=rhs, start=False,
                                         stop=(phase == 2 and bi == 0),
                                         skip_group_check=True)
                if phases is None:
                    acc(0); acc(1); acc(2)
                else:
                    gate_phase[(p, gi, s)] = acc
                    for ph in phases:
                        acc(ph)
                scale = 1.0 if gi == 2 else 0.5

                def act():
                    t = sbA.tile([128, w], BF16,
                                 tag=(f"g{gi}" if s is None else f"g{gi}s{s}"),
                                 name="gt")
                    nc.scalar.activation(out=t, in_=Gs, func=AF.Tanh,
                                         bias=bias_s[:, 1 + gi:2 + gi],
                                         scale=scale)
                    if s is None:
                        gate_sb[p][gi] = t
                    else:
                        gate_sbh[p][gi][s] = t
                return act

            hs_sb = {}

            def emit_update_ew(p, s=None, eng=None):
                """c = 0.5[(tf*cp + cp) + (ti*g + g)]; h = 0.5(1+to)*tanh(c);
                the 0.5s live in the ACT scale and pre-scaled woutT. No PE
                instructions here, so it never blocks the tensor engine;
                eng picks DVE (tail, low latency) or gpsimd (overlapped)."""
                eng = eng or nc.vector
                w = 512 if s is None else 256
                sfx = "" if s is None else f"s{s}"
                if s is None:
                    tf, ti, gg_, to = (gate_sb[p][1], gate_sb[p][0],
                                       gate_sb[p][2], gate_sb[p][3])
                    cp = cprev_sb[p]
                else:
                    tf, ti, gg_, to = (gate_sbh[p][1][s], gate_sbh[p][0][s],
                                       gate_sbh[p][2][s], gate_sbh[p][3][s])
                    cp = cprev_sb[p][:, s * 256:s * 256 + 256]

                def t32(tag):
                    return sbA.tile([128, w], F32, tag=tag + sfx, name=tag)

                def t16(tag):
                    return sbA.tile([128, w], BF16, tag=tag + sfx, name=tag)

                a1 = t32("a1"); eng.tensor_mul(a1, tf, cp)
                a2 = t32("a2"); eng.tensor_add(a2, a1, cp)
                a3 = t16("a3"); eng.tensor_mul(a3, ti, gg_)
                a4 = t16("a4"); eng.tensor_add(a4, a3, gg_)
                wsum = t32("w"); eng.tensor_add(wsum, a2, a4)
                tcs = t16("tc")
                nc.scalar.activation(out=tcs, in_=wsum, func=AF.Tanh, scale=0.5)
                u = t16("u"); eng.tensor_scalar_add(u, to, 1.0)
                hs = t16("h"); eng.tensor_mul(hs, tcs, u)
                hs_sb[(p, s)] = hs

            def emit_update_out(p, s=None, psum=None, mm_only=False,
                                fin_only=False):
                """psum: (tile, colofs) to target a specific PSUM area (the
                tail reuses the retired az banks so the pw pool can stay
                pinned by the gate accumulations)."""
                w = 512 if s is None else 256
                sfx = "" if s is None else f"s{s}"
                if psum is None:
                    OUT = pwp.tile([128, 512], F32, tag="pw", name="OUT")
                    ofs = 0
                else:
                    OUT, ofs = psum
                if not fin_only:
                    hs = hs_sb[(p, s)]
                    nc.tensor.matmul(out=OUT[:, ofs:ofs + w], lhsT=woutT_s,
                                     rhs=hs, start=True, stop=True,
                                     skip_group_check=True)
                if mm_only:
                    return
                osb = sbA.tile([128, w], F32, tag="out" + sfx, name="osb")
                nc.scalar.activation(out=osb, in_=OUT[:, ofs:ofs + w],
                                     func=AF.Identity, bias=bias_s[:, 5:6])
                if s is None:
                    nc.sync.dma_start(
                        out=yout[2 * p:2 * p + 2].rearrange("s c q -> c s q"),
                        in_=osb.rearrange("p (s q) -> p s q", s=2))
                else:
                    nc.sync.dma_start(out=yout[2 * p + s], in_=osb)

            # ---------------- emission schedule ----------------
            # prologue: pass-0 essentials up to branch 3 (pure hc); the
            # vT tiles for the hc source move into iteration-0 filler
            # slots so scores start as early as possible.
            emit_xt(0)
            emit_conv(0, 1)              # hc pass0
            emit_q(0, 3)
            emit_k(0, 3)

            cx0 = conv_chunks(0, 0)      # xc pass0, spread over iter-0 slots
            emit_iter(0, sep=[cx0[0], cx0[1], cx0[2], cx0[3]],
                      fa=[lambda: emit_vt1(0, 1, 0, 0),
                          lambda: emit_vt1(0, 1, 0, 1)],
                      fb=[lambda: emit_vt1(0, 1, 1, 0),
                          lambda: emit_vt1(0, 1, 1, 1)])

            def asyncs():
                emit_cprev(0)
                emit_pads_dma(1)
                emit_cprev(1)

            conv_state = {}

            def emit_conv_part(p, src, part):
                key = (p, src)
                if key not in conv_state:
                    conv_state[key] = conv_chunks(p, src)
                conv_state[key][part]()

            FA = {
                1: [lambda: emit_q(0, 1)],
                2: [lambda: emit_q(0, 2), lambda: emit_vt1(0, 0, 0, 0),
                    asyncs],
                3: [lambda: emit_q(0, 0)],
                4: [lambda: emit_conv_part(1, 1, 0),
                    lambda: emit_conv_part(1, 1, 1),
                    lambda: emit_vt1(0, 0, 1, 0)],
                5: [lambda: emit_conv_part(1, 0, 0),
                    lambda: emit_conv_part(1, 0, 1)],
                6: [lambda: emit_q(1, 3)],
                7: [lambda: emit_q(1, 1), lambda: emit_vt1(1, 1, 0, 0)],
                8: [lambda: emit_q(1, 2)],
                9: [lambda: emit_q(1, 0), lambda: emit_vt1(1, 0, 0, 0)],
                11: [lambda: pending_act.append(gate_mm(0, 0))],
                12: [lambda: pending_act.append(gate_mm(0, 2)),
                     lambda: emit_vt1(1, 1, 1, 1)],
                13: [lambda: emit_update_ew(0, eng=nc.gpsimd),
                     lambda: pending_act.append(gate_mm(1, 0, 0))],
                14: [lambda: pending_act.append(gate_mm(1, 2, 0))],
                15: [lambda: emit_update_out(0)],
            }
            FB = {
                1: [lambda: emit_k(0, 1)],
                2: [lambda: emit_k(0, 2), lambda: emit_vt1(0, 0, 0, 1)],
                3: [lambda: emit_k(0, 0), lambda: emit_xt(1)],
                4: [lambda: emit_conv_part(1, 1, 2),
                    lambda: emit_conv_part(1, 1, 3),
                    lambda: emit_vt1(0, 0, 1, 1)],
                5: [lambda: emit_conv_part(1, 0, 2),
                    lambda: emit_conv_part(1, 0, 3)],
                6: [lambda: emit_k(1, 3)],
                7: [lambda: emit_k(1, 1), lambda: emit_vt1(1, 1, 0, 1)],
                8: [lambda: emit_k(1, 2)],
                9: [lambda: emit_k(1, 0), lambda: emit_vt1(1, 0, 0, 1)],
                11: [lambda: pending_act.append(gate_mm(0, 1)),
                     lambda: emit_vt1(1, 1, 1, 0)],
                12: [lambda: pending_act.append(gate_mm(0, 3)),
                     lambda: emit_vt1(1, 0, 1, 0)],
                13: [lambda: pending_act.append(gate_mm(1, 1, 0)),
                     lambda: emit_vt1(1, 0, 1, 1)],
                14: [lambda: pending_act.append(gate_mm(1, 3, 0))],
            }
            s1act = {}

            for j in range(1, NIT):
                emit_iter(j, fa=FA.get(j, ()), fb=FB.get(j, ()))

            # -------- epilogue: drain the attention pipeline + tail ------
            trs_copies(NIT - 2)
            for f in pending_act:
                f()
            pending_act.clear()
            emit_update_ew(1, 0)        # overlaps the final exp wait
            for g in range(4):
                pv_chunk(NIT - 1, g)
            norm(NIT - 1)
            emit_update_out(1, 0)
            trs_copies(NIT - 1)
            for gi in range(4):
                gate_mm(1, gi, 1)()
            emit_update_ew(1, 1)
            emit_update_out(1, 1)

    nc.compile()
    return nc


def _prep_shared(inputs):
    import concourse.mybir as mybir
    f = np.float32
    bf = mybir.dt.np(mybir.dt.bfloat16)
    c = np.ascontiguousarray
    W_cx, W_ch = np.asarray(inputs["W_cx"], f), np.asarray(inputs["W_ch"], f)
    W_q, W_k, W_v = (np.asarray(inputs[k], f) for k in ("W_q", "W_k", "W_v"))
    W_tok, W_skip = np.asarray(inputs["W_tok"], f), np.asarray(inputs["W_skip"], f)
    b_tok = np.asarray(inputs["b_tok"], f).copy()   # [4, R]
    b_tok[[0, 1, 3]] *= 0.5                          # tanh-as-sigmoid fold
    biases = np.zeros((128, 6), f)
    biases[:, 0] = np.asarray(inputs["b_in"], f)
    biases[:, 1:5] = b_tok.T
    biases[:, 5] = np.asarray(inputs["b_out"], f)
    shared = {
        "winT": c(np.asarray(inputs["W_in"], f).T),
        "biases": biases,
        # [i, src, tap, o]
        "wconvT": c(np.stack([W_cx.transpose(1, 2, 3, 0).reshape(128, 9, 128),
                              W_ch.transpose(1, 2, 3, 0).reshape(128, 9, 128)],
                             axis=1)),
        # [c, (q|k), b, a]
        "wqkT": c(np.stack([W_q.transpose(2, 0, 1), W_k.transpose(2, 0, 1)],
                           axis=1)),
        # [c, srcpair, a-pair]: xc feeds branches (0,2), hc feeds (1,3)
        "wvT": c(np.stack([
            np.concatenate([W_v[0].T, W_v[2].T], axis=1),
            np.concatenate([W_v[1].T, W_v[3].T], axis=1)], axis=1)),
        "ident": np.eye(128, dtype=bf),
        # [a, gate, branch, r]
        "wtokT": c(W_tok.transpose(3, 0, 1, 2)),
        # [c, gate, src, r]
        "wskipT": c(W_skip.transpose(3, 0, 1, 2)),
        # 0.5x from h = 0.5(1+tanh(xo/2)) * tanh(c) folding
        "woutT": c(0.5 * np.asarray(inputs["W_out"], f).T).astype(bf),
    }
    return shared


def kernel(**inputs):
    from concourse.bass_utils import run_bass_kernel_spmd
    if "nc" not in _CACHE:
        _CACHE["nc"] = _build_program()
    nc = _CACHE["nc"]
    f = np.float32
    x = np.asarray(inputs["x"], f).reshape(N, I, HW)
    hp = np.asarray(inputs["h_prev"], f).reshape(N, R, HW)
    cp = np.asarray(inputs["c_prev"], f).reshape(N, R, HW)
    shared = _prep_shared(inputs)
    in_maps = []
    for ci in range(NCORES):
        sl = slice(S * ci, S * ci + S)
        m = dict(shared)
        m["xin"] = np.ascontiguousarray(x[sl])
        m["hin"] = np.ascontiguousarray(hp[sl])
        m["cin"] = np.ascontiguousarray(cp[sl])
        in_maps.append(m)
    res = run_bass_kernel_spmd(nc, in_maps, core_ids=list(range(NCORES)))
    y = np.concatenate([r["yout"].reshape(S, R, H, W) for r in res.results],
                       axis=0)
    return y.astype(np.float32)
